# revision 1
# baseline (speedup 1.0000x reference)
"""Trainium2 Bass kernel for nn_BoneRefusion (17-group BoneMLP over [B,T,16,3]).

Strategy (pure data parallel over batch, 8 cores):
  - Host pre-packs per-core inputs into a feature-major, "2-set" layout:
      xT2 [98, S] bf16, S = tokens_per_core/2.
      Rows 0-47 = 48 features (16 bones x 3 coords) of token set A (first half),
      row 48 = ones (bakes b1 into the layer-1 matmul), rows 49-96 = set B,
      row 97 = ones. Column j holds the token pair (A_j, B_j).
  - Layer 1 (h = relu(x @ W1 + b1)) runs as 5 matmul passes w=0..4 with
    block-diagonal stationary weights [98, Mw] (Mw=128 for w<4: 64 h-features
    x 2 sets; w=4: 16 features x 2 sets). PSUM rows = h features interleaved
    by set. Evacuation PSUM->SBUF applies ReLU and casts to bf16.
  - Layer 2 (out = h @ W2 + b2) runs as column-tiled matmuls (tile_position)
    so 3 of them stream concurrently; b2 is added during PSUM evacuation.
  - Output leaves the device feature-major; the host transposes it back.

All matmuls are bf16 (fp32 matmul is 4x slower on the PE; error measured at
~2e-3 relative on this problem). Accumulation and output are fp32.
"""

import sys

import numpy as np
import ml_dtypes

sys.path.insert(0, "/opt/trn_rl_repo")

import concourse.bass as bass
import concourse.mybir as mybir
import concourse.tile as tile
from concourse import bacc
from concourse.bass_utils import run_bass_kernel_spmd

BF16 = mybir.dt.bfloat16
F32 = mybir.dt.float32
BF16_NP = ml_dtypes.bfloat16

LIMBS = [[0, 1, 2], [3, 4, 5], [6, 7], [8, 9], [10, 11, 12], [13, 14, 15],
         [6, 7, 1, 2], [6, 7, 4, 5], [6, 7, 11, 12], [6, 7, 14, 15], [6, 7, 9],
         [14, 15, 11, 12], [1, 2, 4, 5], [14, 15, 4, 5], [11, 12, 4, 5],
         [10, 0], [13, 3]]
NG = 17          # groups
HID = 16         # hidden per group
B, T, NJ, C = 2048, 243, 16, 3
NF = NJ * C      # 48 input features per token
NCORES = 8
BC = B // NCORES           # batches per core
TC = BC * T                # tokens per core
S = TC // 2                # token pairs per core (2-set packing)
KX = 2 * (NF + 1)          # 98: two sets of (48 features + ones row)
NBLK = 512                 # token-pairs per inner iteration (psum free dim)

# layer-1 passes: groups per pass, features per pass (x2 sets in M)
PASS_GROUPS = [(0, 4), (4, 4), (8, 4), (12, 4), (16, 1)]  # (first group, count)


def _host_weights(W1, b1, W2, b2, idx):
    """Build the stationary operands on the host.

    Returns (w1l [98, 640] bf16, w2l [128, 160] bf16, b2a [96] f32, b2b [56] f32).
    """
    W1 = np.asarray(W1, np.float32)
    b1 = np.asarray(b1, np.float32)
    W2 = np.asarray(W2, np.float32)
    b2 = np.asarray(b2, np.float32)
    idx = np.asarray(idx)

    # Scatter the per-group [12, 16] W1 blocks into the 48-feature space.
    # Padded limb rows of W1 are already zero, so += handles duplicates.
    w1full = np.zeros((NF, NG * HID), np.float32)
    for g in range(NG):
        for j in range(4):
            r = int(idx[g, j]) * C
            w1full[r:r + C, g * HID:(g + 1) * HID] += W1[g, j * C:(j + 1) * C, :]
    b1flat = b1.reshape(NG * HID)

    # Layer-1 stationary tiles, one [98, 128] block per pass (pass 4: [98, 32]).
    w1l = np.zeros((KX, 5 * 128), np.float32)
    for w, (g0, ng) in enumerate(PASS_GROUPS):
        m = ng * HID
        blk = w1full[:, g0 * HID:(g0 + ng) * HID]      # [48, m]
        bias = b1flat[g0 * HID:(g0 + ng) * HID]        # [m]
        col = w * 128
        w1l[0:NF, col:col + m] = blk                   # set A weights
        w1l[NF, col:col + m] = bias                    # set A bias (ones row)
        w1l[NF + 1:2 * NF + 1, col + m:col + 2 * m] = blk   # set B
        w1l[2 * NF + 1, col + m:col + 2 * m] = bias

    # Layer-2 stationary tiles [128, 32] per pass (pass 4 uses rows 0-31).
    # h_sb tile rows: 0..m-1 = set A features, 64..64+m-1 (or m..2m-1 for w=4)
    # = set B features. Output cols: 0..3*ng-1 set A, 12..12+3*ng-1 set B for
    # w<4 (M padded to 32 with zero cols); w=4: cols 0-2 A, 3-5 B.
    w2l = np.zeros((128, 5 * 32), np.float32)
    for w, (g0, ng) in enumerate(PASS_GROUPS):
        col = w * 32
        m = ng * HID
        boff = 64 if w < 4 else m                     # set-B row offset in h tile
        cb = 12 if w < 4 else C * ng                  # set-B col offset
        for j in range(ng):
            g = g0 + j
            w2l[16 * j:16 * j + 16, col + 3 * j:col + 3 * j + 3] = W2[g]
            w2l[boff + 16 * j:boff + 16 * j + 16,
                col + cb + 3 * j:col + cb + 3 * j + 3] = W2[g]

    # Evacuation biases, per psum partition.
    # slot A psum rows: q0: w=0 (rows 0-11 A g0-3, 12-23 B g0-3, 24-31 zero),
    # q1: w=1 (+32), q2: w=2 (+64).  slot B: q0: w=3, q1: w=4 (rows 32-34 A
    # g16, 35-37 B g16).
    b2a = np.zeros(96, np.float32)
    for q, (g0, ng) in enumerate(PASS_GROUPS[:3]):
        v = b2[g0:g0 + ng].reshape(-1)                # 12 values
        b2a[32 * q:32 * q + 12] = v
        b2a[32 * q + 12:32 * q + 24] = v
    b2b = np.zeros(56, np.float32)
    v = b2[12:16].reshape(-1)
    b2b[0:12] = v
    b2b[12:24] = v
    b2b[32:35] = b2[16]
    b2b[35:38] = b2[16]

    return (w1l.astype(BF16_NP), w2l.astype(BF16_NP), b2a[:, None], b2b[:, None])


def _build_nc(repeat=1):
    nc = bacc.Bacc(
        "TRN2", target_bir_lowering=False, debug=False, num_devices=NCORES,
    )
    x2 = nc.dram_tensor("x2", [KX, S], BF16, kind="ExternalInput").ap()
    w1 = nc.dram_tensor("w1", [KX, 5 * 128], BF16, kind="ExternalInput").ap()
    w2 = nc.dram_tensor("w2", [128, 5 * 32], BF16, kind="ExternalInput").ap()
    b2a = nc.dram_tensor("b2a", [96, 1], F32, kind="ExternalInput").ap()
    b2b = nc.dram_tensor("b2b", [56, 1], F32, kind="ExternalInput").ap()
    # Device output, feature-major: slot A rows 0-87 (g0-11 + pad rows) and
    # slot B rows 0-37 (g12-16 + pad rows), concatenated: [126, S].
    outd = nc.dram_tensor("outd", [126, S], F32, kind="ExternalOutput").ap()

    with tile.TileContext(nc) as tc:
        with (
            tc.tile_pool(name="singles", bufs=1) as singles,
            tc.tile_pool(name="xin", bufs=3) as xin,
            tc.tile_pool(name="hsb", bufs=2) as hsb,
            tc.tile_pool(name="osb", bufs=2) as osb,
            tc.tile_pool(name="hps", bufs=1, space="PSUM") as hps,
            tc.tile_pool(name="ops", bufs=1, space="PSUM") as opsp,
        ):
            w1_sb = singles.tile([KX, 5, 128], BF16)
            nc.sync.dma_start(w1_sb, w1.rearrange("k (w m) -> k w m", w=5))
            w2_sb = singles.tile([128, 5, 32], BF16)
            nc.sync.dma_start(w2_sb, w2.rearrange("k (w m) -> k w m", w=5))
            b2a_sb = singles.tile([96, 1], F32)
            nc.sync.dma_start(b2a_sb, b2a)
            b2b_sb = singles.tile([56, 1], F32)
            nc.sync.dma_start(b2b_sb, b2b)

            for _rep in range(repeat):
              off = 0
              while off < S:
                nb = min(NBLK, S - off)
                xt = xin.tile([KX, NBLK], BF16, tag="xt")
                nc.sync.dma_start(xt[:, :nb], x2[:, off:off + nb])

                # ---- layer 1: two double-bank psum tiles + one single ----
                hp01 = hps.tile([128, 2, NBLK], F32, tag="hp01")
                hp23 = hps.tile([128, 2, NBLK], F32, tag="hp23")
                hp4 = hps.tile([32, NBLK], F32, tag="hp4")
                hts = []
                for w in range(5):
                    if w < 4:
                        dst = (hp01 if w < 2 else hp23)[:, w % 2, :nb]
                    else:
                        dst = hp4[:, :nb]
                    mw = 128 if w < 4 else 32
                    nc.tensor.matmul(
                        dst[:mw] if w < 4 else dst,
                        lhsT=w1_sb[:, w, :mw],
                        rhs=xt[:, :nb],
                        start=True, stop=True,
                    )
                # relu evacuations (cast to bf16)
                h01 = hsb.tile([128, 2, NBLK], BF16, tag="h01")
                h23 = hsb.tile([128, 2, NBLK], BF16, tag="h23")
                h4 = hsb.tile([32, NBLK], BF16, tag="h4")
                nc.scalar.activation(
                    out=h01[:, :, :nb], in_=hp01[:, :, :nb],
                    func=mybir.ActivationFunctionType.Relu,
                )
                nc.vector.tensor_scalar(
                    h23[:, :, :nb], hp23[:, :, :nb], 0.0, None,
                    mybir.AluOpType.max,
                )
                nc.vector.tensor_scalar(
                    h4[:, :nb], hp4[:, :nb], 0.0, None,
                    mybir.AluOpType.max,
                )
                hts = [h01[:, 0], h01[:, 1], h23[:, 0], h23[:, 1], h4]

                # ---- layer 2: column-tiled matmuls ----
                oa = opsp.tile([128, NBLK], F32, tag="oa")
                ob = opsp.tile([64, NBLK], F32, tag="ob")
                for w in range(3):
                    nc.tensor.matmul(
                        oa[32 * w:32 * w + 32, :nb],
                        lhsT=w2_sb[:, w, :],
                        rhs=hts[w][:, :nb],
                        start=True, stop=True,
                        tile_position=(0, 32 * w),
                    )
                nc.tensor.matmul(
                    ob[0:32, :nb], lhsT=w2_sb[:, 3, :], rhs=hts[3][:, :nb],
                    start=True, stop=True, tile_position=(0, 0),
                )
                nc.tensor.matmul(
                    ob[32:64, :nb], lhsT=w2_sb[:32, 4, :], rhs=hts[4][:, :nb],
                    start=True, stop=True, tile_position=(0, 32),
                )

                # ---- out evacuations with b2 bias ----
                oas = osb.tile([96, NBLK], F32, tag="oas")
                obs = osb.tile([56, NBLK], F32, tag="obs")
                nc.scalar.activation(
                    out=oas[:, :nb], in_=oa[:96, :nb],
                    func=mybir.ActivationFunctionType.Identity,
                    bias=b2a_sb, scale=1.0,
                )
                nc.vector.tensor_scalar(
                    obs[:, :nb], ob[:56, :nb], b2b_sb, None,
                    mybir.AluOpType.add,
                )

                # ---- store (pad rows included; host slices them off) ----
                nc.sync.dma_start(outd[0:88, off:off + nb], oas[:88, :nb])
                nc.sync.dma_start(outd[88:126, off:off + nb], obs[:38, :nb])

                off += nb
    nc.finalize()
    return nc


def _bench_pair(reps=60, repeats=(1, 5)):
    """Measure kernel time via internal-repeat slope; returns ns per kernel."""
    import time
    import jax
    from jax.sharding import Mesh, PartitionSpec, NamedSharding
    from jax.experimental.shard_map import shard_map
    from concourse import bass2jax
    from concourse.bass2jax import _bass_exec_p, install_neuronx_cc_hook

    install_neuronx_cc_hook()
    rng = np.random.default_rng(0)
    times = {}
    for rep in repeats:
        nc = _build_nc(repeat=rep)
        in_names, out_names, out_avals, zero_outs = [], [], [], []
        for alloc in nc.m.functions[0].allocations:
            if not isinstance(alloc, mybir.MemoryLocationSet):
                continue
            name = alloc.memorylocations[0].name
            if alloc.kind == "ExternalInput":
                if (nc.partition_id_tensor is not None
                        and name == nc.partition_id_tensor.name):
                    continue
                in_names.append(name)
            elif alloc.kind == "ExternalOutput":
                shape = tuple(alloc.tensor_shape)
                dt = mybir.dt.np(alloc.dtype)
                out_avals.append(jax.core.ShapedArray(shape, dt))
                out_names.append(name)
                zero_outs.append(np.zeros(shape, dt))
        n_params, n_outs = len(in_names), len(out_names)
        bind_names = list(in_names) + list(out_names)
        if nc.partition_id_tensor is not None:
            bind_names.append(nc.partition_id_tensor.name)

        def _body(*args, _nc=nc, _oa=tuple(out_avals), _bn=tuple(bind_names),
                  _on=tuple(out_names)):
            operands = list(args)
            if _nc.partition_id_tensor is not None:
                operands.append(bass2jax.partition_id_tensor())
            return tuple(_bass_exec_p.bind(
                *operands, out_avals=_oa, in_names=_bn, out_names=_on,
                lowering_input_output_aliases=(), sim_require_finite=True,
                sim_require_nnan=True, nc=_nc))

        devices = jax.devices()[:NCORES]
        mesh = Mesh(np.asarray(devices), ("core",))
        in_specs = (PartitionSpec("core"),) * (n_params + n_outs)
        out_specs = (PartitionSpec("core"),) * n_outs
        donate = tuple(range(n_params, n_params + n_outs))
        fn = jax.jit(shard_map(_body, mesh=mesh, in_specs=in_specs,
                               out_specs=out_specs, check_rep=False),
                     donate_argnums=donate, keep_unused=True)
        sh = NamedSharding(mesh, PartitionSpec("core"))
        shapes = {"x2": (KX, S), "w1": (KX, 640), "w2": (128, 160),
                  "b2a": (96, 1), "b2b": (56, 1)}
        dts = {"x2": BF16_NP, "w1": BF16_NP, "w2": BF16_NP,
               "b2a": np.float32, "b2b": np.float32}
        concat_in = [jax.device_put(
            rng.normal(size=(NCORES * shapes[nm][0], *shapes[nm][1:])
                       ).astype(dts[nm]) * 0.1, sh) for nm in in_names]

        def make_zeros():
            zs = [jax.device_put(
                np.zeros((NCORES * z.shape[0], *z.shape[1:]), z.dtype), sh)
                for z in zero_outs]
            for a in zs:
                a.block_until_ready()
            return zs

        outs = fn(*concat_in, *make_zeros())
        jax.block_until_ready(outs)
        best = None
        for _trial in range(3):
            zsets = [make_zeros() for _ in range(reps)]
            t0 = time.time()
            for r in range(reps):
                outs = fn(*concat_in, *zsets[r])
            jax.block_until_ready(outs)
            dt = (time.time() - t0) / reps
            best = dt if best is None else min(best, dt)
        times[rep] = best
        print(f"repeat={rep}: {best * 1e6:.1f} us/call", flush=True)
    r0, r1 = repeats
    tk = (times[r1] - times[r0]) / (r1 - r0)
    print(f"kernel time (slope): {tk * 1e9:.0f} ns", flush=True)
    return tk * 1e9


_NC_CACHE = None


def _get_nc():
    global _NC_CACHE
    if _NC_CACHE is None:
        _NC_CACHE = _build_nc()
    return _NC_CACHE


def _kernel_impl(x, W1, b1, W2, b2, idx, _want_trace=False):
    x = np.asarray(x, np.float32)
    w1l, w2l, b2a, b2b = _host_weights(W1, b1, W2, b2, idx)

    in_maps = []
    for c in range(NCORES):
        xc = x[c * BC:(c + 1) * BC].reshape(TC, NF)
        xt2 = np.empty((KX, S), BF16_NP)
        xt2[0:NF] = np.ascontiguousarray(xc[:S].T)
        xt2[NF] = np.float32(1.0)
        xt2[NF + 1:2 * NF + 1] = np.ascontiguousarray(xc[S:].T)
        xt2[2 * NF + 1] = np.float32(1.0)
        in_maps.append({
            "x2": xt2, "w1": w1l, "w2": w2l, "b2a": b2a, "b2b": b2b,
        })

    nc = _get_nc()
    res = run_bass_kernel_spmd(
        nc, in_maps, core_ids=list(range(NCORES)), trace=_want_trace,
    )

    out = np.empty((B, T, NG, C), np.float32)
    for c in range(NCORES):
        od = res.results[c]["outd"]            # [126, S]
        # row map: slot A quarters at 0/32/64 (12 set-A rows then 12 set-B
        # rows each, then 8 pad); slot B at 88 (g12-15) and 120 (g16).
        rows_a = np.r_[0:12, 32:44, 64:76, 88:100, 120:123]
        rows_b = rows_a + np.r_[[12] * 48, [3] * 3]
        oc = np.empty((TC, NG * C), np.float32)
        oc[:S] = od[rows_a].T
        oc[S:] = od[rows_b].T
        out[c * BC:(c + 1) * BC] = oc.reshape(BC, T, NG, C)
    return out, res


def kernel(**inputs):
    out, _ = _kernel_impl(**inputs)
    return out



# revision 11
# speedup vs baseline: 1.5878x; 1.5878x over previous
"""Trainium2 Bass kernel for nn_BoneRefusion (17-group BoneMLP over [B,T,16,3]).

V3 strategy (pure data parallel over batch, 8 cores):

Main pass (groups 0-15 = 256 hidden features/token):
  - Host packs per-core inputs feature-major, 2-set: sbuf tile [112, nb]
    with set A's 48 features at partitions 0-47 and set B's at 64-111.
  - L1 runs as 2 batches (hidden chunks of 128 = groups 0-7 / 8-15), each
    batch = two ROW-TILED matmuls (set A in PE rows 0-63, set B in rows
    64-127) streaming concurrently -> two psum banks [128, nb] per batch.
  - PSUM->SBUF evacuation applies b1 + ReLU (tensor_scalar add/max on
    DVE, activation Relu+bias on ACT), casts to bf16. Work is split
    between Vector and Scalar engines, alternating per block.
  - L2 = one batch of 4 COLUMN-TILED matmuls (M=24 each at col positions
    0/32/64/96): {A g0-7, A g8-15, B g0-7, B g8-15} -> one psum bank.
  - Out evacuation adds b2, casts bf16; DMA out 120 rows (96 useful).

Tail pass (group 16, 16 hidden feats; limbs [13, 3] -> 6 inputs):
  - 16-set packed: K = 8 sets x 6 feats per row-tile (sets 0-7 in PE rows
    0-63, 8-15 in rows 64-127), M = 128 = 8 sets x 16 hidden. Full PE
    utilization for the ragged leftover group instead of poisoning every
    main-loop block with an M=16 pass.
  - L2: two col-tiled matmuls per batch with output column rotation so
    one psum bank collects 2 batches before evacuation.

DMA: all HBM layouts are block-contiguous and grouped 4 blocks per
transfer (4 KB per partition row) to amortize HWDGE descriptor
generation. Output is bf16 (rel-err budget allows it), halving the
biggest HBM stream.

All matmuls bf16; psum fp32; output bf16 -> host casts to fp32.
"""

import sys

import numpy as np
import ml_dtypes

sys.path.insert(0, "/opt/trn_rl_repo")

import concourse.bass as bass
import concourse.mybir as mybir
import concourse.tile as tile
from concourse import bacc
from concourse.bass_utils import run_bass_kernel_spmd

BF16 = mybir.dt.bfloat16
F32 = mybir.dt.float32
BF16_NP = ml_dtypes.bfloat16

LIMBS = [[0, 1, 2], [3, 4, 5], [6, 7], [8, 9], [10, 11, 12], [13, 14, 15],
         [6, 7, 1, 2], [6, 7, 4, 5], [6, 7, 11, 12], [6, 7, 14, 15], [6, 7, 9],
         [14, 15, 11, 12], [1, 2, 4, 5], [14, 15, 4, 5], [11, 12, 4, 5],
         [10, 0], [13, 3]]
NG = 17
HID = 16
B, T, NJ, C = 2048, 243, 16, 3
NF = NJ * C                    # 48 input features per token
NCORES = 8
BC = B // NCORES               # batches per core
TC = BC * T                    # tokens per core (62208)
S = TC // 2                    # token pairs per core (31104)
NBLK = 512                     # token-pairs per block (= one psum bank)
NBLOCKS = (S + NBLK - 1) // NBLK          # 61 (60 full + 384)
GRP = 4                        # blocks per DMA group
NGRPS = (NBLOCKS + GRP - 1) // GRP        # 16
KX = 112                       # sbuf input rows: A 0-47, pad, B 64-111

# ---- tail (group 16) geometry ----
G16_FEATS = [13 * C + 0, 13 * C + 1, 13 * C + 2, 3 * C + 0, 3 * C + 1, 3 * C + 2]
NSETS = 16
TT = TC // NSETS               # tokens per set (3888)
TBLK = 512
NTB = (TT + TBLK - 1) // TBLK  # 8 tail batches (7 full + 272)
TT_LAST = TT - (NTB - 1) * TBLK

# stationary-weight sbuf layout (one [128, 536] bf16 tile):
#   cols 0-127   w1 chunk0 (A rows 0-47, B rows 64-111)
#   cols 128-255 w1 chunk1
#   cols 256-383 w2 main (4 col tiles of M=24 at 256+32j)
#   cols 384-511 w1 tail (8-set block diag, dup at rows 64-111)
#   cols 512-535 w2 tail (8-set block diag, K=128)
WST_COLS = 536


def _host_weights(W1, b1, W2, b2, idx):
    W1 = np.asarray(W1, np.float32)
    b1 = np.asarray(b1, np.float32)
    W2 = np.asarray(W2, np.float32)
    b2 = np.asarray(b2, np.float32)
    idx = np.asarray(idx)

    # dense [48, 272] W1; padded limb rows of W1 are zero so += handles dups
    w1full = np.zeros((NF, NG * HID), np.float32)
    for g in range(NG):
        for j in range(4):
            r = int(idx[g, j]) * C
            w1full[r:r + C, g * HID:(g + 1) * HID] += W1[g, j * C:(j + 1) * C, :]
    b1flat = b1.reshape(NG * HID)

    wst = np.zeros((128, WST_COLS), np.float32)
    # w1 main chunks
    for c in range(2):
        blk = w1full[:, 128 * c:128 * (c + 1)]
        wst[0:48, 128 * c:128 * (c + 1)] = blk
        wst[64:112, 128 * c:128 * (c + 1)] = blk
    # w2 main col tiles: j=0 A g0-7, j=1 A g8-15, j=2 B g0-7, j=3 B g8-15
    for j in range(4):
        g0 = 8 * (j % 2)
        col = 256 + 32 * j
        for g in range(8):
            wst[16 * g:16 * g + 16, col + 3 * g:col + 3 * g + 3] = W2[g0 + g]
    # w1 tail: 8-set block diag of [6, 16], dup at rows 64-111
    w1t = W1[16, 0:6, :]                       # [6, 16]
    for s in range(8):
        wst[6 * s:6 * s + 6, 384 + 16 * s:384 + 16 * s + 16] = w1t
        wst[64 + 6 * s:64 + 6 * s + 6, 384 + 16 * s:384 + 16 * s + 16] = w1t
    # w2 tail: 8-set block diag of [16, 3]
    for s in range(8):
        wst[16 * s:16 * s + 16, 512 + 3 * s:512 + 3 * s + 3] = W2[16]

    # biases, per psum partition: [128, 5] f32
    bias = np.zeros((128, 5), np.float32)
    bias[:, 0] = b1flat[0:128]
    bias[:, 1] = b1flat[128:256]
    boa = np.zeros(128, np.float32)
    for half in range(2):                      # 0 = A (parts 0-63), 1 = B
        for j in range(2):                     # chunk (g0-7 / g8-15)
            base = 64 * half + 32 * j
            boa[base:base + 24] = b2[8 * j:8 * j + 8].reshape(-1)
    bias[:, 2] = boa
    bias[:, 3] = np.tile(b1[16], 8)
    b2t = np.zeros(128, np.float32)
    for m in range(4):
        b2t[32 * m:32 * m + 24] = np.tile(b2[16], 8)
    bias[:, 4] = b2t

    return wst.astype(BF16_NP), bias


def _build_nc():
    nc = bacc.Bacc(
        "TRN2", target_bir_lowering=False, debug=False, num_devices=NCORES,
    )
    x2 = nc.dram_tensor("x2", [NGRPS * KX, GRP * NBLK], BF16,
                        kind="ExternalInput").ap()
    x16 = nc.dram_tensor("x16", [KX, NTB * TBLK], BF16,
                         kind="ExternalInput").ap()
    wst = nc.dram_tensor("wst", [128, WST_COLS], BF16, kind="ExternalInput").ap()
    bias = nc.dram_tensor("bias", [128, 5], F32, kind="ExternalInput").ap()
    outm = nc.dram_tensor("outm", [NGRPS * 120, GRP * NBLK], BF16,
                          kind="ExternalOutput").ap()
    outt = nc.dram_tensor("outt", [120, 4 * TBLK], BF16,
                          kind="ExternalOutput").ap()

    with tile.TileContext(nc) as tc:
        with (
            tc.tile_pool(name="singles", bufs=1) as singles,
            tc.tile_pool(name="xin", bufs=2) as xin,
            tc.tile_pool(name="hsb", bufs=2) as hsb,
            tc.tile_pool(name="osb", bufs=2) as osb,
        ):
            wst_sb = singles.tile([128, WST_COLS], BF16)
            nc.sync.dma_start(wst_sb, wst)
            bias_sb = singles.tile([128, 5], F32)
            nc.sync.dma_start(bias_sb, bias)
            b1c = [bias_sb[:, 0:1], bias_sb[:, 1:2]]
            boa = bias_sb[0:120, 2:3]
            b1t = bias_sb[:, 3:4]
            b2t = bias_sb[0:120, 4:5]

            # ---------------- main pass: groups 0-15 ----------------
            with (
                tc.tile_pool(name="pc0", bufs=2, space="PSUM") as pc0,
                tc.tile_pool(name="pc1", bufs=1, space="PSUM") as pc1,
                tc.tile_pool(name="poa", bufs=2, space="PSUM") as poa,
            ):
                for g in range(NGRPS):
                    xt = xin.tile([KX, GRP * NBLK], BF16, tag="xt")
                    g_blocks = min(GRP, NBLOCKS - g * GRP)
                    g_cols = min(GRP * NBLK, S - g * GRP * NBLK)
                    nc.sync.dma_start(
                        xt[:, :g_cols],
                        x2[g * KX:(g + 1) * KX, :g_cols])
                    ot = osb.tile([120, GRP * NBLK], BF16, tag="ot")

                    for sl in range(g_blocks):
                        b = g * GRP + sl
                        nb = min(NBLK, S - b * NBLK)
                        c0 = sl * NBLK
                        # ---- L1: 2 chunk-batches, row-tiled A/B ----
                        hp0 = pc0.tile([128, 2, NBLK], F32, tag="hp0")
                        hp1 = pc1.tile([128, 2, NBLK], F32, tag="hp1")
                        hp = [hp0, hp1]
                        for cch in range(2):
                            nc.tensor.matmul(
                                hp[cch][:, 0, :nb],
                                lhsT=wst_sb[0:48, 128 * cch:128 * (cch + 1)],
                                rhs=xt[0:48, c0:c0 + nb],
                                start=True, stop=True,
                            )
                            nc.tensor.matmul(
                                hp[cch][:, 1, :nb],
                                lhsT=wst_sb[64:112, 128 * cch:128 * (cch + 1)],
                                rhs=xt[64:112, c0:c0 + nb],
                                start=True, stop=True,
                            )
                        # ---- evac h with bias+relu, split V/S ----
                        ht = hsb.tile([128, 2, 2, NBLK], BF16, tag="ht")
                        ev = [None, None]
                        ev[b % 2] = lambda o, i, s1: nc.vector.tensor_scalar(
                            o, i, s1, 0.0, mybir.AluOpType.add,
                            mybir.AluOpType.max)
                        ev[1 - b % 2] = lambda o, i, s1: nc.scalar.activation(
                            out=o, in_=i,
                            func=mybir.ActivationFunctionType.Relu,
                            bias=s1, scale=1.0)
                        for cch in range(2):
                            ev[cch](ht[:, cch, :, :nb], hp[cch][:, :, :nb],
                                    b1c[cch])
                        # ---- L2: 4 col-tiled matmuls -> one bank ----
                        op = poa.tile([128, NBLK], F32, tag="op")
                        for j in range(4):
                            cch, st = j % 2, j // 2
                            nc.tensor.matmul(
                                op[32 * j:32 * j + 24, :nb],
                                lhsT=wst_sb[:, 256 + 32 * j:256 + 32 * j + 24],
                                rhs=ht[:, cch, st, :nb],
                                start=True, stop=True,
                                tile_position=(0, 32 * j),
                            )
                        # ---- evac out with b2, alternate engine ----
                        if b % 2 == 0:
                            nc.vector.tensor_scalar(
                                ot[:, c0:c0 + nb], op[0:120, :nb], boa, None,
                                mybir.AluOpType.add)
                        else:
                            nc.scalar.activation(
                                out=ot[:, c0:c0 + nb], in_=op[0:120, :nb],
                                func=mybir.ActivationFunctionType.Identity,
                                bias=boa, scale=1.0)
                    nc.sync.dma_start(
                        outm[g * 120:(g + 1) * 120, :g_cols],
                        ot[:, :g_cols])

            # ---------------- tail pass: group 16 ----------------
            with (
                tc.tile_pool(name="pt", bufs=2, space="PSUM") as pt,
                tc.tile_pool(name="pot", bufs=2, space="PSUM") as pot,
                tc.tile_pool(name="x16p", bufs=1) as x16p,
                tc.tile_pool(name="h16p", bufs=2) as h16p,
                tc.tile_pool(name="o16p", bufs=1) as o16p,
            ):
                x16_sb = x16p.tile([KX, NTB * TBLK], BF16)
                half = NTB * TBLK // 2
                nc.sync.dma_start(x16_sb[:, :half], x16[:, :half])
                nc.sync.dma_start(x16_sb[:, half:], x16[:, half:])
                o16 = o16p.tile([120, 4 * TBLK], BF16)
                opt_t = None
                for k in range(NTB):
                    nb = TBLK if k < NTB - 1 else TT_LAST
                    col = k * TBLK
                    htp = pt.tile([128, 2, TBLK], F32, tag="htp")
                    nc.tensor.matmul(
                        htp[:, 0, :nb],
                        lhsT=wst_sb[0:48, 384:512],
                        rhs=x16_sb[0:48, col:col + nb],
                        start=True, stop=True,
                    )
                    nc.tensor.matmul(
                        htp[:, 1, :nb],
                        lhsT=wst_sb[64:112, 384:512],
                        rhs=x16_sb[64:112, col:col + nb],
                        start=True, stop=True,
                    )
                    h16 = h16p.tile([128, 2, TBLK], BF16, tag="h16")
                    if k % 2 == 0:
                        nc.vector.tensor_scalar(
                            h16[:, :, :nb], htp[:, :, :nb], b1t, 0.0,
                            mybir.AluOpType.add, mybir.AluOpType.max)
                    else:
                        nc.scalar.activation(
                            out=h16[:, :, :nb], in_=htp[:, :, :nb],
                            func=mybir.ActivationFunctionType.Relu,
                            bias=b1t, scale=1.0)
                    if k % 2 == 0:
                        opt_t = pot.tile([128, TBLK], F32, tag="opt")
                    ofs = 64 * (k % 2)
                    nc.tensor.matmul(
                        opt_t[ofs:ofs + 24, :nb],
                        lhsT=wst_sb[:, 512:536],
                        rhs=h16[:, 0, :nb],
                        start=True, stop=True,
                        tile_position=(0, ofs),
                    )
                    nc.tensor.matmul(
                        opt_t[ofs + 32:ofs + 56, :nb],
                        lhsT=wst_sb[:, 512:536],
                        rhs=h16[:, 1, :nb],
                        start=True, stop=True,
                        tile_position=(0, ofs + 32),
                    )
                    if k % 2 == 1:
                        q = k // 2
                        if q % 2 == 0:
                            nc.vector.tensor_scalar(
                                o16[:, q * TBLK:(q + 1) * TBLK],
                                opt_t[0:120, :], b2t, None,
                                mybir.AluOpType.add)
                        else:
                            nc.scalar.activation(
                                out=o16[:, q * TBLK:(q + 1) * TBLK],
                                in_=opt_t[0:120, :],
                                func=mybir.ActivationFunctionType.Identity,
                                bias=b2t, scale=1.0)
                nc.sync.dma_start(outt, o16)
    nc.finalize()
    return nc


_NC_CACHE = None


def _get_nc():
    global _NC_CACHE
    if _NC_CACHE is None:
        _NC_CACHE = _build_nc()
    return _NC_CACHE


def _pack_core_inputs(xc):
    """xc: [TC, 48] fp32 for one core -> (x2, x16) bf16 arrays."""
    xa = np.ascontiguousarray(xc[:S].T)        # [48, S]
    xb = np.ascontiguousarray(xc[S:].T)
    x2 = np.zeros((NGRPS, KX, GRP * NBLK), BF16_NP)
    full = NGRPS * GRP * NBLK
    pad = full - S
    xa_p = np.pad(xa, ((0, 0), (0, pad)))
    xb_p = np.pad(xb, ((0, 0), (0, pad)))
    x2[:, 0:48, :] = xa_p.reshape(48, NGRPS, GRP * NBLK).transpose(1, 0, 2)
    x2[:, 64:112, :] = xb_p.reshape(48, NGRPS, GRP * NBLK).transpose(1, 0, 2)

    xg = np.ascontiguousarray(xc[:, G16_FEATS])        # [TC, 6]
    xg = xg.reshape(NSETS, TT, 6)
    x16 = np.zeros((KX, NTB * TBLK), BF16_NP)
    padt = NTB * TBLK - TT
    for s in range(8):
        x16[6 * s:6 * s + 6, :TT] = xg[s].T
        x16[64 + 6 * s:64 + 6 * s + 6, :TT] = xg[8 + s].T
    return x2.reshape(NGRPS * KX, GRP * NBLK), x16


def _unpack_core_output(om, ot_):
    """om: [NGRPS*120, GRP*NBLK] bf16; ot_: [120, 4*TBLK] bf16 ->
    oc [TC, 17, 3] f32."""
    oc = np.empty((TC, NG, C), np.float32)
    om = np.asarray(om, np.float32).reshape(NGRPS, 120, GRP * NBLK)
    om = om.transpose(1, 0, 2).reshape(120, NGRPS * GRP * NBLK)[:, :S]
    for half, t0 in ((0, 0), (1, S)):          # A tokens then B tokens
        for j in range(2):                     # chunk -> groups 8j..8j+7
            rows = om[64 * half + 32 * j:64 * half + 32 * j + 24]
            oc[t0:t0 + S, 8 * j:8 * j + 8, :] = (
                rows.T.reshape(S, 8, C))
    ot_ = np.asarray(ot_, np.float32).reshape(120, 4, TBLK)
    for k in range(NTB):
        nb = TBLK if k < NTB - 1 else TT_LAST
        q, ofs = k // 2, 64 * (k % 2)
        blkcols = ot_[:, q, :nb]               # [120, nb]
        for s8 in range(2):                    # T0 (sets 0-7) / T1 (8-15)
            rows = blkcols[ofs + 32 * s8:ofs + 32 * s8 + 24]   # [24, nb]
            sets = np.arange(8) + 8 * s8
            toks = sets[:, None] * TT + k * TBLK + np.arange(nb)[None, :]
            oc[toks.reshape(-1), 16, :] = (
                rows.reshape(8, C, nb).transpose(0, 2, 1).reshape(-1, C))
    return oc


def _kernel_impl(x, W1, b1, W2, b2, idx, _want_trace=False):
    x = np.asarray(x, np.float32)
    wst, bias = _host_weights(W1, b1, W2, b2, idx)

    in_maps = []
    for c in range(NCORES):
        xc = x[c * BC:(c + 1) * BC].reshape(TC, NF)
        x2, x16 = _pack_core_inputs(xc)
        in_maps.append({"x2": x2, "x16": x16, "wst": wst, "bias": bias})

    nc = _get_nc()
    res = run_bass_kernel_spmd(
        nc, in_maps, core_ids=list(range(NCORES)), trace=_want_trace,
    )

    out = np.empty((B, T, NG, C), np.float32)
    for c in range(NCORES):
        oc = _unpack_core_output(res.results[c]["outm"], res.results[c]["outt"])
        out[c * BC:(c + 1) * BC] = oc.reshape(BC, T, NG, C)
    return out, res


def kernel(**inputs):
    out, _ = _kernel_impl(**inputs)
    return out
